# revision 21
# baseline (speedup 1.0000x reference)
"""GATv2 (2 layers, H=4, C=128, concat=False -> head mean) on 8 Trainium2
NeuronCores, dst-partitioned graph parallel. See bottom for entry point.

Per layer (one SPMD launch of a shared single-layer NEFF):
  dense: xl_hat = x @ (Wl .* |att|) for ALL nodes on every core (f32r matmuls,
         bf16 result to DRAM); xr_hat for the core's local 1280 nodes only.
  gather: per dst block (128 nodes), SWDGE row-gather of xl_hat[src] with
         prepare_only+trigger so transfers stream while engines compute.
         Slots are sorted by src on host for HBM locality.
  edges: per 128-edge chunk pair: xr_hat[dst] via one-hot matmul (fp8
         one-hots), add g via identity matmul (PSUM accum), leaky_relu on
         ACT, p = s*sgn (DVE), logits = per-head reduce (DVE, bf16 out),
         exp on ACT (logits are O(6); segment-max skipped), eg = g*ex with
         one chunk on DVE and one on GPSIMD/Pool, weighted aggregation +
         softmax denominator via one-hot matmuls into PSUM.
  norm:  per block: /denom, undo |att| scaling (+ head mean /4), +bias,
         +residual, mean/var; rstd computed batched at layer end (one
         Ln+Exp pair for all 10 blocks to avoid ACT table swaps), then
         scale + relu + store.
Host does edge sorting/one-hot prep and the inter-layer gather (concat).
"""

from contextlib import ExitStack

import numpy as np
import ml_dtypes

import concourse.bacc as bacc
import concourse.tile as tile
from concourse import mybir
from concourse.bass_utils import run_bass_kernel_spmd

BF16 = ml_dtypes.bfloat16
FP8 = ml_dtypes.float8_e4m3

N_NODES = 10000
D = 128
H = 4
C = 128
HC = H * C
NEG_SLOPE = 0.2
LN_EPS = 1e-5
L = 2

N_CORES = 8
NODES_PER_CORE = 1280
BLOCKS = 10
BLK = 128
N_PAD = N_CORES * NODES_PER_CORE    # 10240
N_ROWS = 10112                      # 79*128
N_TILES = N_ROWS // 128

_NC_CACHE = {}
LAST_RESULTS = []   # BassKernelResults per launch (for test harness)
TRACE_TMPDIRS = None  # optional: test harness sets per-launch tmpdirs

GATHER_PREP = False   # prepare_only+trigger (async) vs blocking dma_gather
OH_FP8 = True         # one-hot / identity matmul lhsT in fp8e4 vs bf16
XLW_SYNC = True       # xl_dram writes issued from SP (True) vs gpsimd


def _prep_edges(edge_index):
    src = np.concatenate([np.asarray(edge_index[0], np.int64),
                          np.arange(N_NODES, dtype=np.int64)])
    dst = np.concatenate([np.asarray(edge_index[1], np.int64),
                          np.arange(N_NODES, dtype=np.int64)])
    pad_nodes = np.arange(N_NODES, N_PAD, dtype=np.int64)
    src = np.concatenate([src, np.zeros_like(pad_nodes)])
    dst = np.concatenate([dst, pad_nodes])

    order = np.argsort(dst, kind="stable")
    src = src[order]
    dst = dst[order]

    blk_of_edge = dst // BLK
    n_blocks_total = N_PAD // BLK
    counts = np.bincount(blk_of_edge, minlength=n_blocks_total)
    K = int(np.max((counts + BLK - 1) // BLK))
    K += K % 2  # even, so we can process chunk pairs

    cap = K * BLK
    src_arr = np.zeros((n_blocks_total, cap), np.int32)
    dpos_arr = np.full((n_blocks_total, cap), -1, np.int32)
    block_starts = np.zeros(n_blocks_total + 1, np.int64)
    np.cumsum(counts, out=block_starts[1:])
    slot = np.arange(len(dst)) - block_starts[blk_of_edge]
    src_arr[blk_of_edge, slot] = src.astype(np.int32)
    dpos_arr[blk_of_edge, slot] = (dst - blk_of_edge * BLK).astype(np.int32)

    # sort slots within each block by src: the SWDGE gather then reads
    # near-consecutive xl rows (HBM row-buffer locality). Pad slots (src 0,
    # dpos -1) sort to the front; harmless row-0 reads.
    sorter = np.argsort(src_arr, axis=1, kind="stable")
    src_arr = np.take_along_axis(src_arr, sorter, axis=1)
    dpos_arr = np.take_along_axis(dpos_arr, sorter, axis=1)

    return (K, src_arr.reshape(N_CORES, BLOCKS, cap),
            dpos_arr.reshape(N_CORES, BLOCKS, cap))


def _build_ship_arrays(K, src_arr, dpos_arr):
    cap = K * BLK
    # wrapped gather indices: idx i lives at [i % 16, i // 16]; the 16-row
    # pattern is tiled 8x along partitions (one copy per SWDGE Q7 core).
    s = src_arr.reshape(N_CORES, BLOCKS, cap // 16, 16)
    s = np.swapaxes(s, 2, 3)                                  # [c,b,16,cap/16]
    sidx = np.tile(s, (1, 1, 8, 1)).astype(np.int16)          # [c,b,128,cap/16]

    # one-hots, laid out partition-major for contiguous DMA, fp8 (0/1 exact):
    # oh [c,b, e(128), k, d(128)], oht [c,b, d(128), k, e(128)]
    ohdt = FP8 if OH_FP8 else BF16
    oh = np.zeros((N_CORES, BLOCKS, BLK, K, BLK), ohdt)
    oht = np.zeros((N_CORES, BLOCKS, BLK, K, BLK), ohdt)
    cc, bb, ss = np.nonzero(dpos_arr >= 0)
    kk = (ss // BLK).astype(np.int64)
    ee = (ss % BLK).astype(np.int64)
    dd = dpos_arr[cc, bb, ss].astype(np.int64)
    oh[cc, bb, ee, kk, dd] = 1
    oht[cc, bb, dd, kk, ee] = 1
    return (np.ascontiguousarray(sidx),
            np.ascontiguousarray(oh.reshape(N_CORES, BLOCKS, BLK, cap)),
            np.ascontiguousarray(oht.reshape(N_CORES, BLOCKS, BLK, cap)))


def _bcast(v, rows=128):
    v = np.asarray(v, np.float32)
    return np.ascontiguousarray(np.broadcast_to(v[None, :], (rows, v.shape[0])))


def _build_nc(K, bias_zero, ln_trivial):
    nc = bacc.Bacc("TRN2", target_bir_lowering=False, debug=False,
                   num_devices=N_CORES)
    f32, bf16, i16 = mybir.dt.float32, mybir.dt.bfloat16, mybir.dt.int16
    f32r = mybir.dt.float32r
    fp8 = mybir.dt.float8e4 if OH_FP8 else mybir.dt.bfloat16
    AF = mybir.ActivationFunctionType
    ALU = mybir.AluOpType
    X = mybir.AxisListType.X
    cap = K * BLK

    xT = nc.dram_tensor("xT", [128, N_ROWS], f32r, kind="ExternalInput")
    xloc = nc.dram_tensor("xloc", [NODES_PER_CORE, 128], f32, kind="ExternalInput")
    xlocT = nc.dram_tensor("xlocT", [128, NODES_PER_CORE], f32r, kind="ExternalInput")
    WlS = nc.dram_tensor("WlS", [128, HC], f32r, kind="ExternalInput")
    WrS = nc.dram_tensor("WrS", [128, HC], f32r, kind="ExternalInput")
    blB = nc.dram_tensor("blB", [128, HC], f32, kind="ExternalInput")
    brB = nc.dram_tensor("brB", [128, HC], f32, kind="ExternalInput")
    sgnB2 = nc.dram_tensor("sgnB2", [128, 2 * HC], bf16, kind="ExternalInput")
    invattB = nc.dram_tensor("invattB", [128, HC], f32, kind="ExternalInput")
    biasB = nc.dram_tensor("biasB", [128, 128], f32, kind="ExternalInput")
    lngB = nc.dram_tensor("lngB", [128, 128], f32, kind="ExternalInput")
    lnbB = nc.dram_tensor("lnbB", [128, 128], f32, kind="ExternalInput")
    ident = nc.dram_tensor("ident", [128, 128], fp8, kind="ExternalInput")
    ohd = nc.dram_tensor("ohd", [BLOCKS, BLK, cap], fp8, kind="ExternalInput")
    ohtd = nc.dram_tensor("ohtd", [BLOCKS, BLK, cap], fp8, kind="ExternalInput")
    sidxd = nc.dram_tensor("sidxd", [BLOCKS, 128, cap // 16], i16,
                           kind="ExternalInput")

    xnew = nc.dram_tensor("xnew", [NODES_PER_CORE, 128], f32,
                          kind="ExternalOutput")

    with tile.TileContext(nc) as tc, ExitStack() as ctx:
        consts = ctx.enter_context(tc.tile_pool(name="consts", bufs=1))
        lhsp = ctx.enter_context(tc.tile_pool(name="lhs", bufs=2))
        densep = ctx.enter_context(tc.tile_pool(name="dense", bufs=3))
        xrp = ctx.enter_context(tc.tile_pool(name="xr", bufs=1))
        gp = ctx.enter_context(tc.tile_pool(name="g", bufs=3))
        ohp = ctx.enter_context(tc.tile_pool(name="oh", bufs=2))
        sxp = ctx.enter_context(tc.tile_pool(name="sx", bufs=3))
        sp = ctx.enter_context(tc.tile_pool(name="s", bufs=3))
        pp = ctx.enter_context(tc.tile_pool(name="p", bufs=3))
        lgp = ctx.enter_context(tc.tile_pool(name="lg", bufs=4))
        exq = ctx.enter_context(tc.tile_pool(name="ex", bufs=4))
        egp = ctx.enter_context(tc.tile_pool(name="eg", bufs=3))
        lnp = ctx.enter_context(tc.tile_pool(name="ln", bufs=2))
        outp = ctx.enter_context(tc.tile_pool(name="out", bufs=2))
        xcp = ctx.enter_context(tc.tile_pool(name="xc", bufs=BLOCKS))
        vsp = ctx.enter_context(tc.tile_pool(name="vs", bufs=1))
        dramp = ctx.enter_context(tc.tile_pool(name="dram", bufs=1, space="DRAM"))
        pzp = ctx.enter_context(tc.tile_pool(name="pz", bufs=2, space="PSUM"))
        paggp = ctx.enter_context(tc.tile_pool(name="pagg", bufs=2, space="PSUM"))
        pdenp = ctx.enter_context(tc.tile_pool(name="pden", bufs=2, space="PSUM"))

        def load_const(src_ap, shape, dtype, name):
            t = consts.tile(shape, dtype, tag=name)
            nc.sync.dma_start(t[:], src_ap)
            return t

        wl_sb = load_const(WlS[:], [128, HC], f32r, "wl")
        wr_sb = load_const(WrS[:], [128, HC], f32r, "wr")
        sgn_sb = load_const(sgnB2[:], [128, 2 * HC], bf16, "sgn")
        invatt_sb = load_const(invattB[:], [128, HC], f32, "invatt")
        lngB_sb = load_const(lngB[:], [128, 128], f32, "lngB")
        lnbB_sb = load_const(lnbB[:], [128, 128], f32, "lnbB")
        id_sb = load_const(ident[:], [128, 128], fp8, "ident")
        if not bias_zero:
            blB_sb = load_const(blB[:], [128, HC], f32, "blB")
            brB_sb = load_const(brB[:], [128, HC], f32, "brB")
            biasB_sb = load_const(biasB[:], [128, 128], f32, "biasB")

        xl_dram = dramp.tile([N_ROWS, HC], bf16)

        epsP = consts.tile([128, 1], f32, tag="epsP")
        nc.vector.memset(epsP[:], LN_EPS)
        alphaP = consts.tile([128, 1], f32, tag="alphaP")
        nc.vector.memset(alphaP[:], NEG_SLOPE)

        # ---- dense: xl_hat for all nodes ----
        # batched lhs loads (8 tiles per DMA); xl writes issued from the
        # gpsimd queue (cheap DMA issue, Pool idle during dense).
        BATCH = 8
        n_batches = (N_TILES + BATCH - 1) // BATCH
        for bt in range(n_batches):
            t0 = bt * BATCH
            t1 = min(t0 + BATCH, N_TILES)
            lhs = lhsp.tile([128, BATCH * 128], f32r, tag="lhs")
            nc.sync.dma_start(lhs[:, :(t1 - t0) * 128],
                              xT[:, t0 * 128:t1 * 128])
            for t_i in range(t0, t1):
                o = (t_i - t0) * 128
                psz = pzp.tile([128, 2 * HC], f32, tag="z2")
                ps = psz[:, :HC]
                nc.tensor.matmul(ps, lhs[:, o:o + 128], wl_sb[:],
                                 start=True, stop=True)
                xs = densep.tile([128, HC], bf16, tag="xs")
                if bias_zero:
                    if t_i % 2 == 0:
                        nc.scalar.activation(xs[:], ps, AF.Copy)
                    else:
                        nc.vector.tensor_copy(xs[:], ps)
                else:
                    nc.vector.tensor_tensor(out=xs[:], in0=ps,
                                            in1=blB_sb[:], op=ALU.add)
                eng = nc.sync if XLW_SYNC else nc.gpsimd
                eng.dma_start(xl_dram[t_i * 128:(t_i + 1) * 128, :], xs[:])

        # ---- dense: xr_hat for local nodes ----
        lhs_xr = lhsp.tile([128, NODES_PER_CORE], f32r, tag="lhsxr")
        nc.sync.dma_start(lhs_xr[:], xlocT[:])
        xr_sb = []
        for b in range(BLOCKS):
            psz = pzp.tile([128, 2 * HC], f32, tag="z2")
            ps = psz[:, :HC]
            nc.tensor.matmul(ps, lhs_xr[:, b * 128:(b + 1) * 128], wr_sb[:],
                             start=True, stop=True)
            t = xrp.tile([128, HC], bf16, tag=f"xr{b}")
            if bias_zero:
                nc.scalar.activation(t[:], ps, AF.Copy)
            else:
                nc.vector.tensor_tensor(out=t[:], in0=ps, in1=brB_sb[:],
                                        op=ALU.add)
            xr_sb.append(t)

        # ---- per-block gathers: prepare_only + trigger so the transfers
        # stream on the DMA engines while all compute engines keep going.
        # Issue 3 ahead (gp bufs=3); later issues happen at the end of each
        # block body so trigger's WAR deps always precede it in-queue. ----
        g_tiles = [None] * BLOCKS
        g_sems = [None] * BLOCKS

        def issue_gather(b):
            six = sxp.tile([128, cap // 16], i16, tag="sidx")
            nc.sync.dma_start(six[:], sidxd[b])
            g = gp.tile([128, K, HC], bf16, tag="g")
            if GATHER_PREP:
                dma_sem = nc.alloc_semaphore(f"gdma{b}")
                prep_sem = nc.alloc_semaphore(f"gprep{b}")
                nc.gpsimd.dma_gather(
                    out_ap=g[:], in_ap=xl_dram[:], idxs_ap=six[:],
                    num_idxs=cap, num_idxs_reg=cap, elem_size=HC,
                    single_packet=False, prepare_only=True,
                    sem=dma_sem).then_inc(prep_sem, 1)
                nc.gpsimd.wait_ge(prep_sem, 1)
                nc.gpsimd.trigger_dma(count=1)
                g_sems[b] = dma_sem
            else:
                nc.gpsimd.dma_gather(
                    out_ap=g[:], in_ap=xl_dram[:], idxs_ap=six[:],
                    num_idxs=cap, num_idxs_reg=cap, elem_size=HC,
                    single_packet=False)
            g_tiles[b] = g

        for b in range(min(3, BLOCKS)):
            issue_gather(b)

        vs_all = vsp.tile([128, 16], f32, tag="vs_all")
        xc_tiles = [None] * BLOCKS
        # ---- edge + norm ----
        for b in range(BLOCKS):
            g = g_tiles[b]
            if GATHER_PREP:
                # tile's auto-sync releases consumers at desc-gen time, not
                # DMA completion; gate every g-consuming engine explicitly.
                nc.tensor.wait_ge(g_sems[b], 16)
                nc.vector.wait_ge(g_sems[b], 16)
                nc.gpsimd.wait_ge(g_sems[b], 16)
            oh = ohp.tile([128, cap], fp8, tag="oh")
            nc.sync.dma_start(oh[:], ohd[b])
            oht = ohp.tile([128, cap], fp8, tag="oht")
            nc.sync.dma_start(oht[:], ohtd[b])

            agg = paggp.tile([128, HC], f32, tag="agg")
            den = pdenp.tile([128, 4], f32, tag="den")

            for kk in range(0, K, 2):
                pz = pzp.tile([128, 2 * HC], f32, tag="z2")
                for j in (0, 1):
                    k = kk + j
                    nc.tensor.matmul(pz[:, j * HC:(j + 1) * HC],
                                     oht[:, k * BLK:(k + 1) * BLK],
                                     xr_sb[b][:], start=True, stop=False)
                    nc.tensor.matmul(pz[:, j * HC:(j + 1) * HC],
                                     id_sb[:], g[:, k, :], start=False, stop=True)
                s = sp.tile([128, 2 * HC], bf16, tag="s")
                nc.scalar.activation(s[:], pz[:], AF.Prelu, alpha=alphaP[:])
                p = pp.tile([128, 2 * HC], bf16, tag="p")
                nc.vector.tensor_tensor(out=p[:], in0=s[:], in1=sgn_sb[:],
                                        op=ALU.mult)
                lg = lgp.tile([128, 8], bf16, tag="lg")
                with nc.allow_low_precision("logits are O(6); bf16 out keeps"
                                            " DVE in 2x mode"):
                    nc.vector.tensor_reduce(
                        out=lg[:], in_=p[:].rearrange("p (g c) -> p g c", c=C),
                        axis=X, op=ALU.add)
                ex = exq.tile([128, 8], bf16, tag="ex")
                nc.scalar.activation(ex[:], lg[:], AF.Exp)
                eg = egp.tile([128, 2 * HC], bf16, tag="eg")
                for j, eng in ((0, nc.vector), (1, nc.gpsimd)):
                    k = kk + j
                    eng.tensor_tensor(
                        out=eg[:, j * HC:(j + 1) * HC]
                            .rearrange("p (h c) -> p h c", h=H),
                        in0=g[:, k, :].rearrange("p (h c) -> p h c", h=H),
                        in1=ex[:, j * 4:(j + 1) * 4].unsqueeze(2)
                            .to_broadcast([128, H, C]),
                        op=ALU.mult)
                for j in (0, 1):
                    k = kk + j
                    nc.tensor.matmul(agg[:], oh[:, k * BLK:(k + 1) * BLK],
                                     eg[:, j * HC:(j + 1) * HC],
                                     start=(k == 0), stop=(k == K - 1))
                    nc.tensor.matmul(den[:], oh[:, k * BLK:(k + 1) * BLK],
                                     ex[:, j * 4:(j + 1) * 4],
                                     start=(k == 0), stop=(k == K - 1))

            rden = lgp.tile([128, 4], f32, tag="rden")
            nc.vector.reciprocal(rden[:], den[:])
            t1 = lnp.tile([128, HC], f32, tag="t1")
            nc.vector.tensor_tensor(out=t1[:], in0=agg[:], in1=invatt_sb[:],
                                    op=ALU.mult)
            t2 = lnp.tile([128, HC], f32, tag="t2")
            nc.gpsimd.tensor_tensor(
                out=t2[:].rearrange("p (h c) -> p h c", h=H),
                in0=t1[:].rearrange("p (h c) -> p h c", h=H),
                in1=rden[:].unsqueeze(2).to_broadcast([128, H, C]),
                op=ALU.mult)
            hm = outp.tile([128, 128], f32, tag="hm")
            nc.vector.tensor_reduce(
                out=hm[:], in_=t2[:].rearrange("p (h c) -> p c h", h=H),
                axis=X, op=ALU.add)
            xt = outp.tile([128, 128], f32, tag="xres")
            nc.sync.dma_start(xt[:], xloc[b * 128:(b + 1) * 128, :])
            if bias_zero:
                r2 = outp.tile([128, 128], f32, tag="r2")
                nc.vector.tensor_tensor(out=r2[:], in0=hm[:], in1=xt[:], op=ALU.add)
            else:
                r1 = outp.tile([128, 128], f32, tag="r1")
                nc.vector.tensor_tensor(out=r1[:], in0=hm[:], in1=biasB_sb[:],
                                        op=ALU.add)
                r2 = outp.tile([128, 128], f32, tag="r2")
                nc.vector.tensor_tensor(out=r2[:], in0=r1[:], in1=xt[:], op=ALU.add)
            mu = lgp.tile([128, 1], f32, tag="mu")
            nc.vector.tensor_reduce(out=mu[:], in_=r2[:], axis=X, op=ALU.add)
            mun = lgp.tile([128, 1], f32, tag="mun")
            nc.vector.tensor_scalar_mul(mun[:], mu[:], 1.0 / 128)
            xc = xcp.tile([128, 128], f32, tag="xc")
            nc.vector.tensor_scalar(out=xc[:], in0=r2[:], scalar1=mun[:],
                                    scalar2=None, op0=ALU.subtract)
            junk = outp.tile([128, 128], f32, tag="junk")
            nc.vector.scalar_tensor_tensor(
                out=junk[:], in0=r2[:], scalar=mun[:], in1=xc[:],
                op0=ALU.subtract, op1=ALU.mult, accum_out=vs_all[:, b:b + 1])
            xc_tiles[b] = xc
            if b + 3 < BLOCKS:
                issue_gather(b + 3)

        # ---- batched rstd: one Ln+Exp pair for all blocks (2 ACT table
        # swaps per layer instead of 2 per block) ----
        lnv = lgp.tile([128, 16], f32, tag="lnv")
        nc.scalar.activation(lnv[:, :BLOCKS], vs_all[:, :BLOCKS], AF.Ln,
                             bias=epsP[:], scale=1.0 / 128)
        rstd = lgp.tile([128, 16], f32, tag="rstd")
        nc.scalar.activation(rstd[:, :BLOCKS], lnv[:, :BLOCKS], AF.Exp,
                             scale=-0.5)
        for b in range(BLOCKS):
            xn = outp.tile([128, 128], f32, tag="xn")
            nc.vector.tensor_scalar(out=xn[:], in0=xc_tiles[b][:],
                                    scalar1=rstd[:, b:b + 1],
                                    scalar2=None, op0=ALU.mult)
            if ln_trivial:
                xgb = xn
            else:
                xg = outp.tile([128, 128], f32, tag="xg")
                nc.vector.tensor_tensor(out=xg[:], in0=xn[:], in1=lngB_sb[:],
                                        op=ALU.mult)
                xgb = outp.tile([128, 128], f32, tag="xgb")
                nc.vector.tensor_tensor(out=xgb[:], in0=xg[:], in1=lnbB_sb[:],
                                        op=ALU.add)
            xout = outp.tile([128, 128], f32, tag="xout")
            nc.scalar.activation(xout[:], xgb[:], AF.Relu)
            nc.sync.dma_start(xnew[b * 128:(b + 1) * 128, :], xout[:])

    nc.compile()
    return nc


def kernel(x, edge_index, Wl, bl, Wr, br, att, bias, ln_g, ln_b):
    x = np.asarray(x, np.float32)
    edge_index = np.asarray(edge_index)
    Wl = np.asarray(Wl, np.float32); bl = np.asarray(bl, np.float32)
    Wr = np.asarray(Wr, np.float32); br = np.asarray(br, np.float32)
    att = np.asarray(att, np.float32); bias = np.asarray(bias, np.float32)
    ln_g = np.asarray(ln_g, np.float32); ln_b = np.asarray(ln_b, np.float32)

    K, src_arr, dpos_arr = _prep_edges(edge_index)
    sidx, oh, oht = _build_ship_arrays(K, src_arr, dpos_arr)

    bias_zero = not (np.any(bias) or np.any(bl) or np.any(br))
    ln_trivial = bool(np.all(ln_g == 1.0) and not np.any(ln_b))
    key = (K, bias_zero, ln_trivial)
    if key not in _NC_CACHE:
        _NC_CACHE[key] = _build_nc(K, bias_zero, ln_trivial)
    nc = _NC_CACHE[key]

    aatt = np.maximum(np.abs(att), 1e-30)
    sgn = np.sign(att).astype(np.float32)
    sgn[sgn == 0] = 1.0

    ident = np.eye(128, dtype=FP8 if OH_FP8 else BF16)
    LAST_RESULTS.clear()
    cur = x
    for l in range(L):
        a_flat = aatt[l].reshape(HC)
        WlS = (Wl[l] * a_flat[None, :]).astype(np.float32)
        WrS = (Wr[l] * a_flat[None, :]).astype(np.float32)
        sgn2 = np.tile(sgn[l].reshape(HC), 2)
        sgnB2 = np.ascontiguousarray(
            np.broadcast_to(sgn2[None, :], (128, 2 * HC))).astype(BF16)

        xpad = np.zeros((N_ROWS, 128), np.float32)
        xpad[:N_NODES] = cur
        xT = np.ascontiguousarray(xpad.T)
        xloc_full = np.zeros((N_PAD, 128), np.float32)
        xloc_full[:N_NODES] = cur

        common = {
            "xT": xT, "WlS": WlS, "WrS": WrS,
            "blB": _bcast(bl[l] * a_flat), "brB": _bcast(br[l] * a_flat),
            "sgnB2": sgnB2, "invattB": _bcast(0.25 / a_flat),
            "biasB": _bcast(bias[l]), "lngB": _bcast(ln_g[l]),
            "lnbB": _bcast(ln_b[l]), "ident": ident,
        }
        in_maps = []
        for c in range(N_CORES):
            xl_c = np.ascontiguousarray(
                xloc_full[c * NODES_PER_CORE:(c + 1) * NODES_PER_CORE])
            in_maps.append({
                **common,
                "xloc": xl_c,
                "xlocT": np.ascontiguousarray(xl_c.T),
                "ohd": oh[c], "ohtd": oht[c], "sidxd": sidx[c],
            })

        td = TRACE_TMPDIRS[l] if TRACE_TMPDIRS else None
        res = run_bass_kernel_spmd(nc, in_maps, core_ids=list(range(N_CORES)),
                                   tmpdir=td)
        LAST_RESULTS.append(res)
        nxt = np.concatenate([res.results[c]["xnew"] for c in range(N_CORES)],
                             axis=0)
        cur = np.ascontiguousarray(nxt[:N_NODES])

    return cur.astype(np.float32)
